# revision 3
# baseline (speedup 1.0000x reference)
"""Depthwise causal Conv1d (B=4, S=4096, D=2048, K=4) on 8 TRN2 NeuronCores.

v5: int8-quantized input AND output, PE diag-matmul + TS-heavy elementwise.

Sharding: 8 cores = batch(4) x sequence-halves(2); zero communication.

Host staging (deterministic, per-channel symmetric int8):
  s_x[d]  = max_s |x[..,d]| / 127
  q       = clip(round(x / s_x), -127, 127)  int8, linearized as
            xin[p, blk*2051 + c] = q[b, s0-3+c, blk*128+p]  (zeros s<0)
  s_o[d]  = (sum_k |w[d,k]|*127*s_x[d] + |b[d]|) / 127   (safe bound: the
            pre-round result can never exceed +-127, so no saturation)
  w'[d,k] = w[d,k]*s_x[d]/s_o[d],  b'[d] = b[d]/s_o[d]
  out     = int8_result * s_o[d]  (host de-staging)

Device per pass (per core), 16 channel blocks in 2 DMA groups of 8:
  in:  SWDGE (gpsimd) dma_start casts DRAM int8 -> SBUF bf16; only 2
       emissions/pass so the Pool Q7 stream is mostly free for its one
       tensor_tensor per EW block (measured 4.06 us each).
  out: HWDGE (sync) int8, issued one group late.
  PE blocks (9): 16 matmuls diag(w'_k) x tap chunks [128,512] accumulated
       into one [128,2048] PSUM tile (4 banks), drained + bias + RNE to
       int8 by one ACT activation per block (~2 us).
  EW blocks (7): ACT m3 = w'3*q3 + b'; DVE TS m2 = w'2*q2 (4x mode);
       DVE TS m1 = w'1*q1; POOL s32 = m3 + m2; DVE TT s321 = s32 + m1;
       DVE STT out_i8 = w'0*q0 + s321.
Measured floors: DMA (cast-in + int8-out) 33 us; PE 30.7; ACT ~31;
DVE ~34; POOL ~30.
"""

import numpy as np

import concourse.bacc as bacc
import concourse.mybir as mybir
from concourse.bass_utils import run_bass_kernel_spmd
from concourse.tile import TileContext

B, S, D, K = 4, 4096, 2048, 4
NCORES = 8
SHALF = S // 2           # 2048 sequence positions per core
HIST = K - 1             # 3 history columns
CW = SHALF + HIST        # 2051 staged columns per block
NBLK = D // 128          # 16 channel blocks
GRP = 8                  # blocks per DMA group
NGRP = NBLK // GRP
CHUNK = 512              # PSUM bank = 512 f32
F32 = mybir.dt.float32
BF16 = mybir.dt.bfloat16
I8 = mybir.dt.int8
MULT = mybir.AluOpType.mult
ADD = mybir.AluOpType.add
IDENT = mybir.ActivationFunctionType.Identity

PE_BLKS = (0, 2, 4, 6, 8, 9, 11, 13, 15)   # 9 PE blocks, 7 EW blocks
PE_IDX = {b: i for i, b in enumerate(PE_BLKS)}

_CACHE = {}


def _build_program(nreps=1, xbufs=3, obufs=2):
    key = ("v5", nreps, GRP, xbufs, obufs)
    if key in _CACHE:
        return _CACHE[key]
    nc = bacc.Bacc("TRN2", num_devices=NCORES)
    x_d = nc.dram_tensor("xin", [128, NBLK * CW], I8, kind="ExternalInput").ap()
    # wtab[p, k*NBLK+blk] = w'[blk*128+p, k] for k<4; wtab[p, 4*NBLK+blk] = b'
    w_d = nc.dram_tensor("wtab", [128, (K + 1) * NBLK], F32, kind="ExternalInput").ap()
    # dwt[p, (pe_idx*K+k)*128 + q] = w'[blk*128+p, k] if p == q else 0
    dw_d = nc.dram_tensor(
        "dwt", [128, len(PE_BLKS) * K * 128], BF16, kind="ExternalInput"
    ).ap()
    o_d = nc.dram_tensor("out", [128, NBLK * SHALF], I8, kind="ExternalOutput").ap()

    with TileContext(nc) as tc:
        with (
            tc.tile_pool(name="const", bufs=1) as const,
            tc.tile_pool(name="xpool", bufs=xbufs) as xpool,
            tc.tile_pool(name="opool", bufs=obufs) as opool,
            tc.tile_pool(name="m3pool", bufs=2) as m3pool,
            tc.tile_pool(name="m2pool", bufs=2) as m2pool,
            tc.tile_pool(name="m1pool", bufs=2) as m1pool,
            tc.tile_pool(name="s32pool", bufs=2) as s32pool,
            tc.tile_pool(name="s321pool", bufs=2) as s321pool,
            tc.tile_pool(name="psum", bufs=2, space="PSUM") as psum,
        ):
            wsb = const.tile([128, (K + 1) * NBLK], F32, tag="wsb")
            nc.scalar.dma_start(out=wsb[:], in_=w_d)
            dwsb = const.tile([128, len(PE_BLKS) * K * 128], BF16, tag="dwsb")
            nc.scalar.dma_start(out=dwsb[:], in_=dw_d)

            def wcol(k, blk):
                return wsb[:, k * NBLK + blk : k * NBLK + blk + 1]

            def dmat(pe_idx, k):
                c0 = (pe_idx * K + k) * 128
                return dwsb[:, c0 : c0 + 128]

            pending = []

            def flush_out(upto):
                while pending and pending[0][0] <= upto:
                    g, tile_ap = pending.pop(0)
                    g %= NGRP
                    nc.sync.dma_start(
                        out=o_d[:, g * GRP * SHALF : (g + 1) * GRP * SHALF],
                        in_=tile_ap,
                    )

            xt = ot = None
            for blk_r in range(NBLK * nreps):
                blk = blk_r % NBLK
                g_r, j = divmod(blk_r, GRP)
                g = g_r % NGRP
                if j == 0:
                    flush_out(g_r - 1)
                    xt = xpool.tile([128, GRP * CW], BF16, tag="xt")
                    nc.gpsimd.dma_start(
                        out=xt[:], in_=x_d[:, g * GRP * CW : (g + 1) * GRP * CW]
                    )
                    ot = opool.tile([128, GRP * SHALF], I8, tag="ot")

                base = j * CW

                def tap(k, lo=0, n=SHALF):
                    return xt[:, base + k + lo : base + k + lo + n]

                osl = ot[:, j * SHALF : (j + 1) * SHALF]

                if blk in PE_IDX:
                    pi = PE_IDX[blk]
                    ps = psum.tile([128, SHALF], F32, tag="ps",
                                   name=f"ps_{blk_r}")
                    for c in range(SHALF // CHUNK):
                        for k in range(K):
                            nc.tensor.matmul(
                                ps[:, c * CHUNK : (c + 1) * CHUNK],
                                dmat(pi, k),
                                tap(k, c * CHUNK, CHUNK),
                                start=(k == 0),
                                stop=(k == K - 1),
                            )
                    nc.scalar.activation(
                        osl, ps[:], IDENT, bias=wcol(K, blk), scale=1.0
                    )
                else:
                    m3 = m3pool.tile([128, SHALF], BF16, tag="m3",
                                     name=f"m3_{blk_r}")
                    nc.scalar.activation(
                        m3[:], tap(3), IDENT, bias=wcol(K, blk), scale=wcol(3, blk)
                    )
                    m2 = m2pool.tile([128, SHALF], BF16, tag="m2",
                                     name=f"m2_{blk_r}")
                    nc.vector.tensor_scalar(
                        out=m2[:], in0=tap(2), scalar1=wcol(2, blk),
                        scalar2=None, op0=MULT,
                    )
                    s32 = s32pool.tile([128, SHALF], BF16, tag="s32",
                                       name=f"s32_{blk_r}")
                    nc.gpsimd.tensor_tensor(out=s32[:], in0=m3[:], in1=m2[:], op=ADD)
                    m1 = m1pool.tile([128, SHALF], BF16, tag="m1",
                                     name=f"m1_{blk_r}")
                    nc.vector.tensor_scalar(
                        out=m1[:], in0=tap(1), scalar1=wcol(1, blk),
                        scalar2=None, op0=MULT,
                    )
                    s321 = s321pool.tile([128, SHALF], BF16, tag="s321",
                                         name=f"s321_{blk_r}")
                    nc.vector.tensor_tensor(out=s321[:], in0=s32[:], in1=m1[:], op=ADD)
                    nc.vector.scalar_tensor_tensor(
                        osl, tap(0), wcol(0, blk), s321[:], MULT, ADD
                    )

                if j == GRP - 1:
                    pending.append((g_r, ot[:]))
            flush_out(NGRP * nreps)

    nc.compile()
    _CACHE[key] = nc
    return nc


def _bf16_dtype():
    import ml_dtypes

    return np.dtype(ml_dtypes.bfloat16)


def _to_bf16(a):
    return np.asarray(a, dtype=np.float32).astype(_bf16_dtype())


def _quant_scales(x, weight, bias):
    """Per-channel input scale s_x and output scale s_o (safe bound)."""
    ax = np.abs(x).max(axis=(0, 1))                       # [D] max|x| per channel
    ax = np.maximum(ax, 1e-30)
    s_x = ax / 127.0
    om_bound = np.abs(weight).sum(axis=1) * 127.0 * s_x + np.abs(bias)
    s_o = np.maximum(om_bound, 1e-30) / 127.0
    return s_x, s_o


def _shard_inputs(x, weight, bias):
    x = np.asarray(x, dtype=np.float32)
    weight = np.asarray(weight, dtype=np.float32)[:, 0, :]   # [D, K]
    bias = np.asarray(bias, dtype=np.float32)

    s_x, s_o = _quant_scales(x, weight, bias)
    wq = weight * (s_x / s_o)[:, None]                       # folded taps [D, K]
    bq = bias / s_o                                          # folded bias [D]

    wr = wq.reshape(NBLK, 128, K)                            # [blk, p, k]
    wtab = np.empty((128, (K + 1) * NBLK), dtype=np.float32)
    wtab[:, : K * NBLK] = wr.transpose(1, 2, 0).reshape(128, K * NBLK)
    wtab[:, K * NBLK :] = bq.reshape(NBLK, 128).T

    dwt = np.zeros((128, len(PE_BLKS) * K * 128), dtype=np.float32)
    rng = np.arange(128)
    for pi, blk in enumerate(PE_BLKS):
        for k in range(K):
            dwt[rng, (pi * K + k) * 128 + rng] = wr[blk, :, k]
    dwt = _to_bf16(dwt)

    q = np.clip(np.round(x / s_x[None, None, :]), -127, 127).astype(np.int8)

    in_maps = []
    for core in range(NCORES):
        b, h = divmod(core, 2)
        s0 = h * SHALF
        xc = np.empty((NBLK, 128, CW), dtype=np.int8)
        xbt = q[b].T.reshape(NBLK, 128, S)  # [blk, p, s] view
        if s0 == 0:
            xc[:, :, :HIST] = 0
            xc[:, :, HIST:] = xbt[:, :, :SHALF]
        else:
            xc[:] = xbt[:, :, s0 - HIST : s0 + SHALF]
        xin = np.ascontiguousarray(xc.transpose(1, 0, 2)).reshape(128, NBLK * CW)
        in_maps.append({"xin": xin, "wtab": wtab, "dwt": dwt})
    return in_maps, s_o


def _run(x, weight, bias, trace=False):
    nc = _build_program()
    in_maps, s_o = _shard_inputs(x, weight, bias)
    res = run_bass_kernel_spmd(nc, in_maps, list(range(NCORES)), trace=trace)
    # de-stage: int8 [128, NBLK*SHALF] -> f32 [SHALF, D] per core
    s_o_staged = s_o.reshape(NBLK, 128)                      # [blk, p]
    out = np.empty((B, S, D), dtype=np.float32)
    for core in range(NCORES):
        b, h = divmod(core, 2)
        o = np.asarray(res.results[core]["out"]).astype(np.float32)
        o = o.reshape(128, NBLK, SHALF) * s_o_staged.T[:, :, None]
        o = o.transpose(1, 0, 2).reshape(D, SHALF)
        out[b, h * SHALF : (h + 1) * SHALF, :] = o.T
    return out, res


def kernel(x, weight, bias):
    out, _ = _run(x, weight, bias, trace=False)
    return out
